# revision 1
# baseline (speedup 1.0000x reference)
"""Trainium2 Bass kernel: pre-norm decoder block (B=2, S=2048, D=1024, H=16, DFF=4096).

Sharding: 8 cores = 2 data-parallel groups (one per batch) x 4 tensor-parallel
ranks. Within a group, attention is head-sharded (4 heads/core, Megatron-style
column-parallel QKV). Instead of Megatron's post-Wo all-reduce, each rank's
normalized per-head attention output (transposed layout [256, 2048]) is
AllGathered group-locally; the rest of the block (Wo + residual + LN2 + FFN +
residual) runs sequence-sharded: each rank processes its own 512 sequence
positions with the full Wo/w1/w2, so no further collectives are needed.

All on-chip activations live in transposed layout ([feature, seq]) so every
matmul's output layout matches what the next matmul wants (no on-chip
transposes). LayerNorm statistics are ones-vector matmuls on the tensor
engine (partition-axis reduction in this layout); the per-position LN affine
is folded into the projections via extra contraction rows.
"""

import numpy as np

import concourse.bass as bass
import concourse.mybir as mybir
import concourse.tile as tile
from concourse import bacc
from concourse.bass_utils import run_bass_kernel_spmd

F32R = mybir.dt.float32r
F32 = mybir.dt.float32
U32 = mybir.dt.uint32
AF = mybir.ActivationFunctionType
ALU = mybir.AluOpType

B, S, D, H, DFF = 2, 2048, 1024, 16, 4096
DH = D // H
EPS = 1e-5

N_CORES = 8
TP = 4                    # tensor-parallel ranks per group
HC = H // TP              # heads per core
DC = HC * DH              # head features per core
RC = S // TP              # seq rows per core in stage B
FT = D // 128             # feature tiles
SB = S // 512             # 512-wide seq blocks
MT = DFF // 128           # dff tiles
REPLICA_GROUPS = [[0, 1, 2, 3], [4, 5, 6, 7]]


def build(debug=False, repeat=1, qkv_bias=False):
    assert not (debug and repeat > 1)
    nc = bacc.Bacc("TRN2", target_bir_lowering=False, debug=False,
                   num_devices=N_CORES)

    d = {"qkv_bias": qkv_bias}
    d["xt"] = nc.dram_tensor("xt", [128, FT, S], F32R, kind="ExternalInput")
    d["xres"] = nc.dram_tensor("xres", [128, FT, RC], F32R, kind="ExternalInput")
    d["wq"] = nc.dram_tensor("wq", [2, 128, FT, 128], F32R, kind="ExternalInput")
    d["wk"] = nc.dram_tensor("wk", [2, 128, FT, 128], F32R, kind="ExternalInput")
    d["wv"] = nc.dram_tensor("wv", [128, FT, DC], F32R, kind="ExternalInput")
    d["qkvc"] = nc.dram_tensor("qkvc", [6, DC], F32R, kind="ExternalInput")
    d["wo"] = nc.dram_tensor("wo", [128, FT, FT, 128], F32R, kind="ExternalInput")
    d["w1"] = nc.dram_tensor("w1", [MT, 128, FT, 128], F32R, kind="ExternalInput")
    d["b1t"] = nc.dram_tensor("b1t", [128, MT], F32, kind="ExternalInput")
    d["w2"] = nc.dram_tensor("w2", [FT, 128, MT, 128], F32R, kind="ExternalInput")
    d["b2t"] = nc.dram_tensor("b2t", [128, FT], F32, kind="ExternalInput")
    d["mask4"] = nc.dram_tensor("mask4", [4, 128, 512], F32R, kind="ExternalInput")
    d["recipd"] = nc.dram_tensor("recipd", [128, 2], F32R, kind="ExternalInput")
    d["ones64"] = nc.dram_tensor("ones64", [128, 16, HC, 1], F32R, kind="ExternalInput")
    d["colsel"] = nc.dram_tensor("colsel", [1, 1], U32, kind="ExternalInput")
    d["out"] = nc.dram_tensor("out", [128, FT, RC], F32, kind="ExternalOutput")
    if debug:
        d["dbg_qt"] = nc.dram_tensor("dbg_qt", [128, 2, S], F32, kind="ExternalOutput")
        d["dbg_kt"] = nc.dram_tensor("dbg_kt", [128, 2, S], F32, kind="ExternalOutput")
        d["dbg_v"] = nc.dram_tensor("dbg_v", [128, 16, HC, 65], F32, kind="ExternalOutput")
        d["dbg_at"] = nc.dram_tensor("dbg_at", [128, 2, S], F32, kind="ExternalOutput")
        d["dbg_stats"] = nc.dram_tensor("dbg_stats", [1, 3 * S], F32, kind="ExternalOutput")
        d["dbg_h"] = nc.dram_tensor("dbg_h", [128, FT, RC], F32, kind="ExternalOutput")
        d["dbg_hn"] = nc.dram_tensor("dbg_hn", [128, FT, RC], F32, kind="ExternalOutput")
        d["dbg_ag"] = nc.dram_tensor("dbg_ag", [128, FT, RC], F32, kind="ExternalOutput")

    with tile.TileContext(nc) as tc:
        for _ in range(repeat):
            _emit(nc, tc, d, debug)

    nc.compile()
    return nc


def _emit(nc, tc, d, debug):
    qkv_bias = d["qkv_bias"]
    with (
        tc.tile_pool(name="dram", bufs=1, space="DRAM") as dramp,
        tc.tile_pool(name="w1s", bufs=2) as w1p,
        tc.tile_pool(name="outer", bufs=1) as outp,
    ):
        bounce_in = dramp.tile([2 * 128, S], F32R, tag="bounce_in")
        bounce_out = dramp.tile([TP * 2 * 128, S], F32R, tag="bounce_out")
        recipd = outp.tile([128, 2], F32R, tag="recipd")
        nc.sync.dma_start(out=recipd[:], in_=d["recipd"].ap())

        # ============================ stage A ============================
        with tc.tile_pool(name="persa", bufs=1) as pa:
            qt_sb = pa.tile([128, 2, S], F32R, tag="qt")
            kt_sb = pa.tile([128, 2, S], F32R, tag="kt")
            v_sb = pa.tile([128, 16, HC, 65], F32R, tag="v")
            nc.sync.dma_start(out=v_sb[:, :, :, 64:65], in_=d["ones64"].ap())

            with tc.tile_pool(name="xpool", bufs=1) as xp:
                x_sb = xp.tile([128, FT, S], F32R, tag="x")
                nc.sync.dma_start(out=x_sb[:], in_=d["xt"].ap())
                wq_sb = xp.tile([128, 2, FT, 128], F32R, tag="wq")
                wk_sb = xp.tile([128, 2, FT, 128], F32R, tag="wk")
                wv_sb = xp.tile([128, FT, DC], F32R, tag="wv")
                for dd in range(2):
                    nc.sync.dma_start(out=wq_sb[:, dd], in_=d["wq"].ap()[dd])
                    nc.sync.dma_start(out=wk_sb[:, dd], in_=d["wk"].ap()[dd])
                nc.sync.dma_start(out=wv_sb[:], in_=d["wv"].ap())
                stats = xp.tile([1, 3 * S], F32R, tag="stats")
                MU, M2, VAR = 0, S, 2 * S
                LNV = M2  # lnv overwrites m2 (no longer needed)
                rs_row = xp.tile([1, S], F32R, tag="rs_row")
                nmu_row = xp.tile([1, S], F32R, tag="nmu_row")
                std_row = xp.tile([1, S], F32R, tag="std_row") if qkv_bias else None
                a_b = xp.tile([128, S], F32R, tag="a_b")
                rst = xp.tile([128, S // 128], F32R, tag="rst")
                ncst = 6 if qkv_bias else 3
                cst = [xp.tile([1, DC], F32R, tag=f"qkvc{i}", name=f"qkvc{i}")
                       for i in range(ncst)]
                for i in range(ncst):
                    nc.sync.dma_start(out=cst[i][:], in_=d["qkvc"].ap()[i:i + 1, :])
                wqs, wks, wvs = cst[0][:], cst[1][:], cst[2][:]
                if qkv_bias:
                    bqc, bkc, bvc = cst[3][:], cst[4][:], cst[5][:]
                else:
                    bqc = bkc = bvc = None

                # ---- LN1 statistics --------------------------------
                with (
                    tc.tile_pool(name="stps", bufs=4, space="PSUM") as stps,
                    tc.tile_pool(name="sq", bufs=3) as sqp,
                ):
                    st = [stps.tile([2, 512], F32, tag="stmu", name=f"stmu{_s}")
                          for _s in range(SB)]
                    stm2 = [stps.tile([2, 512], F32, tag="stm2", name=f"stm2{_s}")
                            for _s in range(SB)]
                    for f in range(FT):
                        for s in range(SB):
                            sl = bass.ts(s, 512)
                            x2 = sqp.tile([128, 512], F32R, tag="x2")
                            nc.scalar.activation(x2[:], x_sb[:, f, sl], AF.Square)
                            nc.tensor.matmul(st[s][0:2, :], recipd[:], x_sb[:, f, sl],
                                             start=(f == 0), stop=(f == FT - 1))
                            nc.tensor.matmul(stm2[s][0:2, :], recipd[:], x2[:],
                                             start=(f == 0), stop=(f == FT - 1))
                    for s in range(SB):
                        sl = bass.ts(s, 512)
                        nc.scalar.copy(stats[:, MU + 512 * s:MU + 512 * s + 512], st[s][0:1, :])
                        nc.scalar.copy(stats[:, M2 + 512 * s:M2 + 512 * s + 512], stm2[s][0:1, :])
                nc.vector.tensor_tensor(stats[:, VAR:VAR + S], stats[:, MU:MU + S],
                                        stats[:, MU:MU + S], ALU.mult)
                nc.vector.scalar_tensor_tensor(stats[:, VAR:VAR + S],
                                               stats[:, M2:M2 + S], EPS,
                                               stats[:, VAR:VAR + S],
                                               op0=ALU.add, op1=ALU.subtract)
                nc.scalar.activation(stats[:, LNV:LNV + S], stats[:, VAR:VAR + S],
                                     AF.Ln)
                nc.scalar.activation(rs_row[:], stats[:, LNV:LNV + S],
                                     AF.Exp, scale=-0.5)
                if qkv_bias:
                    nc.scalar.activation(std_row[:], stats[:, LNV:LNV + S],
                                         AF.Exp, scale=0.5)
                nc.vector.tensor_scalar(out=nmu_row[:], in0=stats[:, MU:MU + S],
                                        scalar1=-1.0, scalar2=None, op0=ALU.mult)
                nc.gpsimd.partition_broadcast(a_b[:], rs_row[:])
                drs = dramp.tile([1, S], F32R, tag="drs")
                nc.sync.dma_start(out=drs[:], in_=rs_row[:])
                nc.sync.dma_start(
                    out=rst[:],
                    in_=drs[:].rearrange("o (t p) -> (o p) t", p=128))
                if debug:
                    nc.sync.dma_start(out=d["dbg_stats"].ap(),
                                      in_=stats[:].bitcast(F32))

                # ---- projections -----------------------------------
                with tc.tile_pool(name="prps", bufs=2, space="PSUM") as prps:
                    for (w_sb, wsum, bc, o_sb) in ((wq_sb, wqs, bqc, qt_sb),
                                                   (wk_sb, wks, bkc, kt_sb)):
                        for dd in range(2):
                            dsl = bass.ts(dd, 128)
                            for s in range(SB):
                                sl = bass.ts(s, 512)
                                ps = prps.tile([128, 512], F32, tag="pp")
                                for f in range(FT):
                                    nc.tensor.matmul(ps[:], w_sb[:, dd, f],
                                                     x_sb[:, f, sl],
                                                     start=(f == 0), stop=False)
                                nc.tensor.matmul(ps[:], wsum[0:1, dsl],
                                                 nmu_row[:, sl], start=False,
                                                 stop=not qkv_bias)
                                if qkv_bias:
                                    nc.tensor.matmul(ps[:], bc[0:1, dsl],
                                                     std_row[:, sl],
                                                     start=False, stop=True)
                                nc.vector.tensor_tensor(o_sb[:, dd, sl], ps[:],
                                                        a_b[:, sl], ALU.mult)
                    for i in range(16):
                        rl = bass.ts(i, 128)
                        ps = prps.tile([128, DC], F32, tag="pv")
                        for f in range(FT):
                            nc.tensor.matmul(ps[:], x_sb[:, f, rl], wv_sb[:, f],
                                             start=(f == 0), stop=False)
                        nc.tensor.matmul(ps[:], nmu_row[:, rl], wvs,
                                         start=False, stop=not qkv_bias)
                        if qkv_bias:
                            nc.tensor.matmul(ps[:], std_row[:, rl], bvc,
                                             start=False, stop=True)
                        nc.vector.tensor_scalar(
                            out=v_sb[:, i, :, 0:64],
                            in0=ps[:].rearrange("p (h e) -> p h e", h=HC),
                            scalar1=rst[:, i:i + 1].bitcast(F32), scalar2=None,
                            op0=ALU.mult)
            # x pool closed

            if debug:
                nc.sync.dma_start(out=d["dbg_qt"].ap(), in_=qt_sb[:].bitcast(F32))
                nc.sync.dma_start(out=d["dbg_kt"].ap(), in_=kt_sb[:].bitcast(F32))
                nc.sync.dma_start(out=d["dbg_v"].ap(), in_=v_sb[:].bitcast(F32))

            # ---- attention -----------------------------------------
            with (
                tc.tile_pool(name="attp", bufs=1) as atp_a,
                tc.tile_pool(name="scps", bufs=2, space="PSUM") as scps,
                tc.tile_pool(name="pvps", bufs=4, space="PSUM") as pvps,
                tc.tile_pool(name="exps", bufs=6) as expp,
                tc.tile_pool(name="rcps", bufs=3) as rcpp,
            ):
                attnt_sb = atp_a.tile([128, 2, S], F32R, tag="attnt")
                mask4 = atp_a.tile([128, 4, 512], F32R, tag="mask4")
                nc.sync.dma_start(out=mask4[:],
                                  in_=d["mask4"].ap().rearrange("j p c -> p j c"))
                for hp in range(2):
                    heads = (2 * hp, 2 * hp + 1)
                    for qi in range(SB):
                        qsl = bass.ts(qi, 512)
                        nki = 4 * qi + 4
                        pv = {h: pvps.tile([65, 512], F32, tag="pv",
                                           name=f"pv{h}_{qi}") for h in heads}
                        for ki in range(nki):
                            for h in heads:
                                hb = 64 * (h % 2)
                                hs = slice(hb, hb + 64)
                                sc = scps.tile([128, 512], F32, tag="sc")
                                nc.tensor.matmul(
                                    sc[:], kt_sb[hs, h // 2, bass.ts(ki, 128)],
                                    qt_sb[hs, h // 2, qsl], start=True, stop=True)
                                ex = expp.tile([128, 512], F32R, tag="ex")
                                nc.scalar.activation(ex[:], sc[:], AF.Exp)
                                rel = 128 * ki - 512 * qi
                                if rel >= 0:
                                    mw = rel + 128
                                    nc.vector.tensor_tensor(
                                        ex[:, 0:mw], ex[:, 0:mw],
                                        mask4[:, rel // 128, 0:mw], ALU.mult)
                                nc.tensor.matmul(pv[h][:], v_sb[:, ki, h, :],
                                                 ex[:], start=(ki == 0),
                                                 stop=(ki == nki - 1))
                        for h in heads:
                            hb = 64 * (h % 2)
                            rcp = rcpp.tile([1, 512], F32R, tag="rcp")
                            rcpb = rcpp.tile([64, 512], F32R, tag="rcpb")
                            with nc.allow_low_precision(reason="fp32r bits"):
                                nc.vector.reciprocal(rcp[:], pv[h][64:65, :])
                            nc.gpsimd.partition_broadcast(rcpb[:], rcp[:])
                            nc.vector.tensor_tensor(
                                attnt_sb[hb:hb + 64, h // 2, qsl],
                                pv[h][0:64, :], rcpb[:], ALU.mult)

                if debug:
                    nc.sync.dma_start(out=d["dbg_at"].ap(),
                                      in_=attnt_sb[:].bitcast(F32))
                nc.sync.dma_start(
                    out=bounce_in[:].rearrange("(c p) s -> p c s", p=128),
                    in_=attnt_sb[:])
        # stage-A pools closed

        nc.gpsimd.collective_compute(
            "AllGather", ALU.bypass, replica_groups=REPLICA_GROUPS,
            ins=[bounce_in.opt()], outs=[bounce_out.opt()])

        # ============================ stage B ============================
        creg = nc.alloc_registers(f"colsel_r_{nc.next_id()}")
        nc.regs_load(creg, d["colsel"].ap()[0:1, 0:1])
        colsv = nc.snap(creg, donate=True, min_val=0, max_val=S - RC)

        with tc.tile_pool(name="persb", bufs=1) as pb:
            h_sb = pb.tile([128, FT, RC], F32R, tag="h")
            out_sb = pb.tile([128, FT, RC], F32, tag="outt")
            st2 = pb.tile([1, 4 * RC], F32R, tag="st2")
            MU2, M22, VAR2, LNV2 = 0, RC, 2 * RC, 3 * RC
            rs2_row = pb.tile([1, RC], F32R, tag="rs2_row")
            l2b = pb.tile([1, RC], F32R, tag="l2b")
            l2a_b = pb.tile([128, RC], F32R, tag="l2a_b")
            l2b_b = pb.tile([128, RC], F32R, tag="l2b_b")
            bias_sb = pb.tile([128, MT + FT], F32, tag="bias")
            nc.sync.dma_start(out=bias_sb[:, 0:MT], in_=d["b1t"].ap())
            nc.sync.dma_start(out=bias_sb[:, MT:MT + FT], in_=d["b2t"].ap())

            # ---- Wo + residual -------------------------------------
            with (
                tc.tile_pool(name="wotmp", bufs=1) as wop,
                tc.tile_pool(name="agp", bufs=2) as agp,
                tc.tile_pool(name="atin", bufs=1) as atp,
                tc.tile_pool(name="wops", bufs=2, space="PSUM") as wops,
            ):
                at_in = atp.tile([128, FT, RC], F32R, tag="at_in")
                xres_sb = atp.tile([128, FT, RC], F32R, tag="xres")
                nc.sync.dma_start(out=xres_sb[:], in_=d["xres"].ap())
                bo_view = bounce_out[:].rearrange("(f p) s -> p f s", p=128)
                for f in range(FT):
                    agt = agp.tile([128, S], F32R, tag="agt")
                    nc.sync.dma_start(out=agt[:], in_=bo_view[:, f, :])
                    nc.scalar.copy(at_in[:, f, :], agt[:, bass.ds(colsv, RC)])
                if debug:
                    nc.sync.dma_start(out=d["dbg_ag"].ap(),
                                      in_=at_in[:].bitcast(F32))
                wo_sb = wop.tile([128, FT, FT, 128], F32R, tag="wo")
                nc.sync.dma_start(out=wo_sb[:], in_=d["wo"].ap())
                for dd in range(FT):
                    ps = wops.tile([128, RC], F32, tag="wops")
                    for f in range(FT):
                        nc.tensor.matmul(ps[:], wo_sb[:, f, dd], at_in[:, f, :],
                                         start=(f == 0), stop=(f == FT - 1))
                    nc.vector.tensor_add(h_sb[:, dd, :], ps[:], xres_sb[:, dd, :])
            if debug:
                nc.sync.dma_start(out=d["dbg_h"].ap(), in_=h_sb[:].bitcast(F32))

            # ---- LN2 + FFN -----------------------------------------
            with (
                tc.tile_pool(name="hnp", bufs=1) as hnp,
                tc.tile_pool(name="ap_", bufs=1) as ap_,
                tc.tile_pool(name="w2s", bufs=2) as w2p,
                tc.tile_pool(name="sq2", bufs=2) as sq2p,
                tc.tile_pool(name="st2ps", bufs=1, space="PSUM") as st2ps,
                tc.tile_pool(name="f1ps", bufs=3, space="PSUM") as f1ps,
                tc.tile_pool(name="f2ps", bufs=2, space="PSUM") as f2ps,
            ):
                stp = st2ps.tile([2, RC], F32, tag="st2p")
                stp2 = st2ps.tile([2, RC], F32, tag="st2p2")
                for f in range(FT):
                    h2 = sq2p.tile([128, RC], F32R, tag="h2")
                    nc.scalar.activation(h2[:], h_sb[:, f], AF.Square)
                    nc.tensor.matmul(stp[0:2, :], recipd[:], h_sb[:, f, :],
                                     start=(f == 0), stop=(f == FT - 1))
                    nc.tensor.matmul(stp2[0:2, :], recipd[:], h2[:],
                                     start=(f == 0), stop=(f == FT - 1))
                nc.scalar.copy(st2[:, MU2:MU2 + RC], stp[0:1, :])
                nc.scalar.copy(st2[:, M22:M22 + RC], stp2[0:1, :])
                nc.vector.tensor_tensor(st2[:, VAR2:VAR2 + RC], st2[:, MU2:MU2 + RC],
                                        st2[:, MU2:MU2 + RC], ALU.mult)
                nc.vector.scalar_tensor_tensor(st2[:, VAR2:VAR2 + RC],
                                               st2[:, M22:M22 + RC], EPS,
                                               st2[:, VAR2:VAR2 + RC],
                                               op0=ALU.add, op1=ALU.subtract)
                nc.scalar.activation(st2[:, LNV2:LNV2 + RC], st2[:, VAR2:VAR2 + RC],
                                     AF.Ln)
                nc.scalar.activation(rs2_row[:], st2[:, LNV2:LNV2 + RC],
                                     AF.Exp, scale=-0.5)
                nc.vector.scalar_tensor_tensor(l2b[:], st2[:, MU2:MU2 + RC], -1.0,
                                               rs2_row[:],
                                               op0=ALU.mult, op1=ALU.mult)
                nc.gpsimd.partition_broadcast(l2a_b[:], rs2_row[:])
                nc.gpsimd.partition_broadcast(l2b_b[:], l2b[:])

                hn_sb = hnp.tile([128, FT, RC], F32R, tag="hn")
                for f in range(FT):
                    nc.vector.tensor_tensor(hn_sb[:, f, :], h_sb[:, f, :],
                                            l2a_b[:], ALU.mult)
                    nc.vector.tensor_add(hn_sb[:, f, :], hn_sb[:, f, :], l2b_b[:])
                if debug:
                    nc.sync.dma_start(out=d["dbg_hn"].ap(),
                                      in_=hn_sb[:].bitcast(F32))

                a_sb = ap_.tile([128, MT, RC], F32R, tag="a")
                for m in range(MT):
                    w1m = w1p.tile([128, FT, 128], F32R, tag="w1m")
                    nc.sync.dma_start(out=w1m[:], in_=d["w1"].ap()[m])
                    ps = f1ps.tile([128, RC], F32, tag="f1")
                    for f in range(FT):
                        nc.tensor.matmul(ps[:], w1m[:, f, :], hn_sb[:, f, :],
                                         start=(f == 0), stop=(f == FT - 1))
                    nc.scalar.activation(a_sb[:, m, :], ps[:], AF.Relu,
                                         bias=bias_sb[:, m:m + 1])

                for dd in range(FT):
                    w2d = w2p.tile([128, MT, 128], F32R, tag="w2d")
                    nc.sync.dma_start(out=w2d[:], in_=d["w2"].ap()[dd])
                    ps = f2ps.tile([128, RC], F32, tag="f2")
                    for t in range(MT):
                        nc.tensor.matmul(ps[:], w2d[:, t, :], a_sb[:, t, :],
                                         start=(t == 0), stop=(t == MT - 1))
                    nc.vector.scalar_tensor_tensor(
                        out_sb[:, dd, :], ps[:],
                        bias_sb[:, MT + dd:MT + dd + 1],
                        h_sb[:, dd, :], op0=ALU.add, op1=ALU.add)
            nc.sync.dma_start(out=d["out"].ap(), in_=out_sb[:])


# ----------------------------------------------------------------------
# host side
# ----------------------------------------------------------------------

def make_in_maps(x, mask, Wq, Wk, Wv, Wo, w1, b1, w2, b2, g1, be1, g2, be2):
    """Build the 8 per-core input maps from the full inputs."""
    f32 = np.float32
    x = np.asarray(x, f32)
    mask = np.asarray(mask)
    Wq, Wk, Wv, Wo = (np.asarray(t, f32) for t in (Wq, Wk, Wv, Wo))
    w1, b1, w2, b2 = (np.asarray(t, f32) for t in (w1, b1, w2, b2))
    g1, be1, g2, be2 = (np.asarray(t, f32) for t in (g1, be1, g2, be2))

    Wq_s = g1[:, None] * Wq / np.sqrt(np.float32(DH))
    Wk_s = g1[:, None] * Wk
    Wv_s = g1[:, None] * Wv
    bq_full = (be1 @ Wq) / np.sqrt(np.float32(DH))
    bk_full = be1 @ Wk
    bv_full = be1 @ Wv
    w1_s = g2[:, None] * w1
    b1_s = b1 + be2 @ w1
    m2d = np.asarray(mask[0, 0], bool)
    mask4 = np.stack([m2d[0:512, 128 * j:128 * j + 128].T.astype(f32)
                      for j in range(4)])
    recipd = np.full((128, 2), 1.0 / D, f32)
    ones64 = np.ones((128, 16, HC, 1), f32)
    b1t = np.ascontiguousarray(b1_s.reshape(MT, 128).T)
    b2t = np.ascontiguousarray(b2.reshape(FT, 128).T)
    wo_p = np.ascontiguousarray(Wo.reshape(FT, 128, FT, 128).transpose(1, 0, 2, 3))
    w1_p = np.ascontiguousarray(w1_s.reshape(FT, 128, MT, 128).transpose(2, 1, 0, 3))
    w2_p = np.ascontiguousarray(w2.reshape(MT, 128, FT, 128).transpose(2, 1, 0, 3))

    in_maps = []
    for c in range(N_CORES):
        g, r = divmod(c, TP)
        xT = np.ascontiguousarray(x[g].T)                       # [D, S]
        xt = np.ascontiguousarray(xT.reshape(FT, 128, S).transpose(1, 0, 2))
        xres = np.ascontiguousarray(
            xT[:, RC * r:RC * (r + 1)].reshape(FT, 128, RC).transpose(1, 0, 2))
        sh = slice(DC * r, DC * (r + 1))
        wq_c = np.ascontiguousarray(
            Wq_s[:, sh].reshape(FT, 128, 2, 128).transpose(2, 1, 0, 3))
        wk_c = np.ascontiguousarray(
            Wk_s[:, sh].reshape(FT, 128, 2, 128).transpose(2, 1, 0, 3))
        wv_c = np.ascontiguousarray(
            Wv_s[:, sh].reshape(FT, 128, DC).transpose(1, 0, 2))
        qkvc = np.stack([Wq_s[:, sh].sum(0), Wk_s[:, sh].sum(0),
                         Wv_s[:, sh].sum(0), bq_full[sh], bk_full[sh],
                         bv_full[sh]]).astype(f32)
        in_maps.append({
            "xt": xt, "xres": xres, "wq": wq_c, "wk": wk_c, "wv": wv_c,
            "qkvc": qkvc, "wo": wo_p, "w1": w1_p, "b1t": b1t, "w2": w2_p,
            "b2t": b2t, "mask4": mask4, "recipd": recipd,
            "ones64": ones64,
            "colsel": np.array([[RC * r]], np.uint32),
        })
    return in_maps


def assemble_output(results):
    """[8 x {out: [128, FT, RC]}] -> [B, S, D] float32."""
    out = np.empty((B, S, D), np.float32)
    for c in range(N_CORES):
        g, r = divmod(c, TP)
        ot = results[c]["out"].transpose(1, 0, 2).reshape(D, RC)  # [D, RC]
        out[g, RC * r:RC * (r + 1), :] = ot.T
    return out


_nc_cache = {}


def get_nc(debug=False, repeat=1, qkv_bias=False):
    key = (debug, repeat, qkv_bias)
    if key not in _nc_cache:
        _nc_cache[key] = build(debug=debug, repeat=repeat, qkv_bias=qkv_bias)
    return _nc_cache[key]


def kernel(**inputs):
    qkv_bias = bool(np.any(np.asarray(inputs["be1"], np.float32)))
    nc = get_nc(qkv_bias=qkv_bias)
    in_maps = make_in_maps(**inputs)
    res = run_bass_kernel_spmd(nc, in_maps, core_ids=list(range(N_CORES)))
    return assemble_output(res.results)



# revision 10
# speedup vs baseline: 1.0916x; 1.0916x over previous
"""Trainium2 Bass kernel: pre-norm decoder block (B=2, S=2048, D=1024, H=16, DFF=4096).

Sharding: 8 cores = 2 data-parallel groups (one per batch) x 4 tensor-parallel
ranks. Attention is head-sharded (4 heads/core, Megatron column-parallel QKV).
Each rank computes its partial Wo contribution (row-parallel Wo) per 512-wide
sequence block as attention for that block completes; a single bf16
ReduceScatter over the sequence axis then hands every rank the fully-reduced
pre-residual h for its own 512 rows. The rest (residual + LN2 + FFN +
residual) runs sequence-sharded with full w1/w2 (no further collectives).

Everything on-chip is bf16 (psum accumulation fp32): same tensor-engine speed
as fp32r at these tile sizes, but half the DMA/SBUF footprint and 2-4x DVE
throughput. LayerNorm statistics are computed by DVE pairwise-reduction trees
plus a single ones-vector matmul per 512-column block; the per-position LN
affine is folded into the projections via extra contraction rows (LN1) or a
broadcasted scale/shift (LN2).
"""

import numpy as np
import ml_dtypes

import concourse.bass as bass
import concourse.mybir as mybir
import concourse.tile as tile
from concourse import bacc
from concourse.bass_utils import run_bass_kernel_spmd

BF = mybir.dt.bfloat16
F32 = mybir.dt.float32
AF = mybir.ActivationFunctionType
ALU = mybir.AluOpType

B, S, D, H, DFF = 2, 2048, 1024, 16, 4096
DH = D // H
EPS = 1e-5

N_CORES = 8
TP = 4                    # tensor-parallel ranks per group
HC = H // TP              # heads per core
DC = HC * DH              # head features per core
RC = S // TP              # seq rows per core in stage B
FT = D // 128             # feature tiles
SB = S // 512             # 512-wide seq blocks
MT = DFF // 128           # dff tiles
REPLICA_GROUPS = [[0, 1, 2, 3], [4, 5, 6, 7]]


def build(repeat=1, qkv_bias=False):
    nc = bacc.Bacc("TRN2", target_bir_lowering=False, debug=False,
                   num_devices=N_CORES)

    d = {"qkv_bias": qkv_bias}
    d["xt"] = nc.dram_tensor("xt", [128, FT, S], BF, kind="ExternalInput")
    d["xres"] = nc.dram_tensor("xres", [128, FT, RC], BF, kind="ExternalInput")
    d["wq"] = nc.dram_tensor("wq", [2, 128, FT, 128], BF, kind="ExternalInput")
    d["wk"] = nc.dram_tensor("wk", [2, 128, FT, 128], BF, kind="ExternalInput")
    d["wv"] = nc.dram_tensor("wv", [128, FT, DC], BF, kind="ExternalInput")
    d["qkvc"] = nc.dram_tensor("qkvc", [6, DC], BF, kind="ExternalInput")
    d["wo"] = nc.dram_tensor("wo", [128, 2, FT, 128], BF, kind="ExternalInput")
    d["w1"] = nc.dram_tensor("w1", [128, MT, FT, 128], BF, kind="ExternalInput")
    d["b1t"] = nc.dram_tensor("b1t", [128, MT], F32, kind="ExternalInput")
    d["w2"] = nc.dram_tensor("w2", [128, MT, FT, 128], BF, kind="ExternalInput")
    d["b2t"] = nc.dram_tensor("b2t", [128, FT], F32, kind="ExternalInput")
    d["mask4"] = nc.dram_tensor("mask4", [4, 128, 512], BF, kind="ExternalInput")
    d["onesp"] = nc.dram_tensor("onesp", [128, 1], BF, kind="ExternalInput")
    d["ones64"] = nc.dram_tensor("ones64", [128, 16, HC, 1], BF, kind="ExternalInput")
    d["out"] = nc.dram_tensor("out", [128, FT, RC], F32, kind="ExternalOutput")

    with tile.TileContext(nc) as tc:
        for _ in range(repeat):
            _emit(nc, tc, d)

    nc.compile()
    return nc


def _emit(nc, tc, d):
    qkv_bias = d["qkv_bias"]
    with (
        tc.tile_pool(name="dram", bufs=1, space="DRAM") as dramp,
        tc.tile_pool(name="const", bufs=1) as cp,
        tc.tile_pool(name="w1p", bufs=1) as w1p,
    ):
        # DRAM bounce buffers for the collective
        wob = dramp.tile([SB, 128, FT, RC], BF, tag="wob")
        hb = dramp.tile([128, FT, RC], BF, tag="hb")
        drs = dramp.tile([1, S], F32, tag="drs")

        # persistent small tensors + weight prefetch (no dependencies)
        onesp = cp.tile([128, 1], BF, tag="onesp")
        b1t = cp.tile([128, MT], F32, tag="b1t")
        b2t = cp.tile([128, FT], F32, tag="b2t")
        rst = cp.tile([128, 16], F32, tag="rst")
        ncst = 6 if qkv_bias else 3
        cst = [cp.tile([1, DC], BF, tag=f"qkvc{i}", name=f"qkvc{i}")
               for i in range(ncst)]
        w1_sb = w1p.tile([128, MT, FT, 128], BF, tag="w1")

        nc.sync.dma_start(out=onesp[:], in_=d["onesp"].ap())
        for i in range(ncst):
            nc.sync.dma_start(out=cst[i][:], in_=d["qkvc"].ap()[i:i + 1, :])
        nc.sync.dma_start(out=b1t[:], in_=d["b1t"].ap())
        nc.sync.dma_start(out=b2t[:], in_=d["b2t"].ap())
        wqs, wks, wvs = cst[0][:], cst[1][:], cst[2][:]
        if qkv_bias:
            bqc, bkc, bvc = cst[3][:], cst[4][:], cst[5][:]

        with tc.tile_pool(name="persa", bufs=1) as pa:
            qt_sb = pa.tile([128, 2, S], BF, tag="qt")
            kt_sb = pa.tile([128, 2, S], BF, tag="kt")
            v_sb = pa.tile([128, 16, HC, 65], BF, tag="v")
            mask4 = pa.tile([128, 4, 512], BF, tag="mask4")
            wo_sb = pa.tile([128, 2, FT, 128], BF, tag="wo")
            nc.sync.dma_start(out=mask4[:],
                              in_=d["mask4"].ap().rearrange("j p c -> p j c"))

            # ================= phase 1: LN1 stats + projections ==========
            with (
                tc.tile_pool(name="xpool", bufs=1) as xp,
                tc.tile_pool(name="sq", bufs=1) as sqp,
                tc.tile_pool(name="vtp", bufs=2) as vtp,
                tc.tile_pool(name="stps", bufs=2, space="PSUM") as stps,
                tc.tile_pool(name="prps", bufs=3, space="PSUM") as prps,
            ):
                x_sb = xp.tile([128, FT, S], BF, tag="x")
                wq_sb = xp.tile([128, 2, FT, 128], BF, tag="wq")
                wk_sb = xp.tile([128, 2, FT, 128], BF, tag="wk")
                wv_sb = xp.tile([128, FT, DC], BF, tag="wv")
                nmu_row = xp.tile([1, S], BF, tag="nmu_row")
                std_row = xp.tile([1, S], BF, tag="std_row") if qkv_bias else None
                rs_row = xp.tile([1, S], F32, tag="rs_row")
                a_b = xp.tile([128, S], F32, tag="a_b")

                # x first (critical path), then stage-A weights, then
                # stage-B weights (wo tiny, w1 big) — all on the sync queue.
                for b in range(SB):
                    sl = bass.ts(b, 512)
                    nc.sync.dma_start(out=x_sb[:, :, sl],
                                      in_=d["xt"].ap()[:, :, sl])
                for dd in range(2):
                    nc.sync.dma_start(out=wq_sb[:, dd], in_=d["wq"].ap()[dd])
                    nc.sync.dma_start(out=wk_sb[:, dd], in_=d["wk"].ap()[dd])
                nc.sync.dma_start(out=wv_sb[:], in_=d["wv"].ap())
                nc.sync.dma_start(out=v_sb[:, :, :, 64:65], in_=d["ones64"].ap())
                nc.sync.dma_start(out=wo_sb[:], in_=d["wo"].ap())
                nc.sync.dma_start(out=w1_sb[:], in_=d["w1"].ap())

                def emit_stats(b):
                    sl = bass.ts(b, 512)
                    # DVE pairwise trees for sum(x) and sum(x^2) over the
                    # 8 feature tiles, then one ones-vector matmul for the
                    # partition reduction.
                    x2 = sqp.tile([128, FT, 512], BF, tag="x2")
                    t4 = sqp.tile([128, 4, 512], BF, tag="t4")
                    sums = sqp.tile([128, 2, 512], BF, tag="sums")
                    nc.vector.tensor_tensor(x2[:], x_sb[:, :, sl], x_sb[:, :, sl],
                                            ALU.mult)
                    nc.vector.tensor_tensor(t4[:], x_sb[:, 0:4, sl],
                                            x_sb[:, 4:8, sl], ALU.add)
                    nc.vector.tensor_tensor(t4[:, 0:2], t4[:, 0:2], t4[:, 2:4],
                                            ALU.add)
                    nc.vector.tensor_tensor(sums[:, 0], t4[:, 0], t4[:, 1],
                                            ALU.add)
                    nc.vector.tensor_tensor(t4[:], x2[:, 0:4], x2[:, 4:8],
                                            ALU.add)
                    nc.vector.tensor_tensor(t4[:, 0:2], t4[:, 0:2], t4[:, 2:4],
                                            ALU.add)
                    nc.vector.tensor_tensor(sums[:, 1], t4[:, 0], t4[:, 1],
                                            ALU.add)
                    stp = stps.tile([1, 2, 512], F32, tag="stp")
                    nc.tensor.matmul(stp[:, 0], onesp[:], sums[:, 0],
                                     start=True, stop=True)
                    nc.tensor.matmul(stp[:, 1], onesp[:], sums[:, 1],
                                     start=True, stop=True)
                    # row chain: nmu = -sum(x)/D ; var = sum(x2)/D - mu^2 ;
                    # rs = exp(-0.5*ln(var+eps)) ; a_b = broadcast(rs)
                    vt = vtp.tile([1, 2, 512], F32, tag="vt")
                    nc.vector.tensor_scalar(out=nmu_row[:, sl], in0=stp[0:1, 0],
                                            scalar1=-1.0 / D, scalar2=None,
                                            op0=ALU.mult)
                    nc.vector.tensor_tensor(vt[:, 0], nmu_row[:, sl],
                                            nmu_row[:, sl], ALU.mult)
                    nc.vector.tensor_scalar(out=vt[:, 0], in0=vt[:, 0],
                                            scalar1=EPS, scalar2=None,
                                            op0=ALU.subtract)
                    nc.vector.scalar_tensor_tensor(vt[:, 1], stp[0:1, 1],
                                                   1.0 / D, vt[:, 0],
                                                   op0=ALU.mult,
                                                   op1=ALU.subtract)
                    nc.scalar.activation(vt[:, 0], vt[:, 1], AF.Ln)
                    nc.scalar.activation(rs_row[:, sl], vt[:, 0], AF.Exp,
                                         scale=-0.5)
                    if qkv_bias:
                        nc.scalar.activation(std_row[:, sl], vt[:, 0], AF.Exp,
                                             scale=0.5)
                    nc.gpsimd.partition_broadcast(a_b[:, sl], rs_row[:, sl])
                    # rs for v-scaling needs a [128, 4] column layout: bounce
                    # through DRAM with a rearranging read.
                    nc.scalar.dma_start(out=drs[:, sl], in_=rs_row[:, sl])
                    nc.gpsimd.dma_start(
                        out=rst[:, 4 * b:4 * b + 4],
                        in_=drs[:, sl].rearrange("o (t p) -> (o p) t", p=128))

                def emit_proj(b):
                    sl = bass.ts(b, 512)
                    for (w_sb, wsum, bc, o_sb) in (
                            (wq_sb, wqs, bqc if qkv_bias else None, qt_sb),
                            (wk_sb, wks, bkc if qkv_bias else None, kt_sb)):
                        for dd in range(2):
                            dsl = bass.ts(dd, 128)
                            ps = prps.tile([128, 512], F32, tag="pp")
                            for f in range(FT):
                                nc.tensor.matmul(ps[:], w_sb[:, dd, f],
                                                 x_sb[:, f, sl],
                                                 start=(f == 0), stop=False)
                            nc.tensor.matmul(ps[:], wsum[0:1, dsl],
                                             nmu_row[:, sl], start=False,
                                             stop=not qkv_bias)
                            if qkv_bias:
                                nc.tensor.matmul(ps[:], bc[0:1, dsl],
                                                 std_row[:, sl],
                                                 start=False, stop=True)
                            nc.vector.tensor_tensor(o_sb[:, dd, sl], ps[:],
                                                    a_b[:, sl], ALU.mult)
                    for i in range(4 * b, 4 * b + 4):
                        rl = bass.ts(i, 128)
                        ps = prps.tile([128, 512], F32, tag="pp")
                        for f in range(FT):
                            nc.tensor.matmul(ps[:, 0:DC], x_sb[:, f, rl],
                                             wv_sb[:, f],
                                             start=(f == 0), stop=False)
                        nc.tensor.matmul(ps[:, 0:DC], nmu_row[:, rl], wvs,
                                         start=False, stop=not qkv_bias)
                        if qkv_bias:
                            nc.tensor.matmul(ps[:, 0:DC], std_row[:, rl], bvc,
                                             start=False, stop=True)
                        nc.vector.tensor_scalar(
                            out=v_sb[:, i, :, 0:64],
                            in0=ps[:, 0:DC].rearrange("p (h e) -> p h e", h=HC),
                            scalar1=rst[:, i:i + 1], scalar2=None,
                            op0=ALU.mult)

                # software-pipelined by one block so the DVE never
                # head-of-line blocks the next block's stats tree
                emit_stats(0)
                for b in range(1, SB):
                    emit_stats(b)
                    emit_proj(b - 1)
                emit_proj(SB - 1)
            # xpool closed

            # ================= phase 2: attention + Wo partials ==========
            with (
                tc.tile_pool(name="atq", bufs=2) as atqp,
                tc.tile_pool(name="wos", bufs=2) as wosp,
                tc.tile_pool(name="exps", bufs=8) as expp,
                tc.tile_pool(name="rcps", bufs=4) as rcpp,
                tc.tile_pool(name="scps", bufs=4, space="PSUM") as scps,
                tc.tile_pool(name="pvps", bufs=4, space="PSUM") as pvps,
            ):
                for qi in range(SB):
                    qsl = bass.ts(qi, 512)
                    attn_qi = atqp.tile([128, 2, 512], BF, tag="attq")
                    for hp in range(2):
                        heads = (2 * hp, 2 * hp + 1)
                        nki = 4 * qi + 4
                        pv = {h: pvps.tile([65, 512], F32, tag="pv",
                                           name=f"pv{h}_{qi}") for h in heads}
                        for ki in range(nki):
                            for h in heads:
                                hb_ = 64 * (h % 2)
                                hs = slice(hb_, hb_ + 64)
                                sc = scps.tile([128, 512], F32, tag="sc")
                                nc.tensor.matmul(
                                    sc[:], kt_sb[hs, hp, bass.ts(ki, 128)],
                                    qt_sb[hs, hp, qsl], start=True, stop=True)
                                ex = expp.tile([128, 512], BF, tag="ex")
                                nc.scalar.activation(ex[:], sc[:], AF.Exp)
                                rel = 128 * ki - 512 * qi
                                if rel >= 0:
                                    mw = rel + 128
                                    nc.vector.tensor_tensor(
                                        ex[:, 0:mw], ex[:, 0:mw],
                                        mask4[:, rel // 128, 0:mw], ALU.mult)
                                nc.tensor.matmul(pv[h][:], v_sb[:, ki, h, :],
                                                 ex[:], start=(ki == 0),
                                                 stop=(ki == nki - 1))
                        for h in heads:
                            hb_ = 64 * (h % 2)
                            rcp = rcpp.tile([1, 512], BF, tag="rcp")
                            rcpb = rcpp.tile([64, 512], BF, tag="rcpb")
                            with nc.allow_low_precision(reason="bf16 softmax"):
                                nc.vector.reciprocal(rcp[:], pv[h][64:65, :])
                            nc.gpsimd.partition_broadcast(rcpb[:], rcp[:])
                            nc.vector.tensor_tensor(
                                attn_qi[hb_:hb_ + 64, hp, :],
                                pv[h][0:64, :], rcpb[:], ALU.mult)
                    # row-parallel Wo partial for this seq block -> bounce
                    wops = wosp.tile([128, FT, 512], BF, tag="wops")
                    for dd in range(FT):
                        ps = scps.tile([128, 512], F32, tag="sc")
                        nc.tensor.matmul(ps[:], wo_sb[:, 0, dd],
                                         attn_qi[:, 0, :],
                                         start=True, stop=False)
                        nc.tensor.matmul(ps[:], wo_sb[:, 1, dd],
                                         attn_qi[:, 1, :],
                                         start=False, stop=True)
                        nc.vector.tensor_copy(out=wops[:, dd, :], in_=ps[:])
                    nc.gpsimd.dma_start(out=wob[qi], in_=wops[:])

                # one bf16 ReduceScatter over the seq axis: rank r receives
                # sum_ranks(partial h) for its own 512 rows
                nc.gpsimd.collective_compute(
                    "ReduceScatter", ALU.add, replica_groups=REPLICA_GROUPS,
                    ins=[wob[:].opt()], outs=[hb[:].opt()])
        # persa closed

        # ================= phase 3: residual + LN2 + FFN =================
        with (
            tc.tile_pool(name="pb", bufs=1) as pb,
            tc.tile_pool(name="sq2", bufs=1) as sq2p,
            tc.tile_pool(name="outp", bufs=2) as outp,
            tc.tile_pool(name="st2ps", bufs=1, space="PSUM") as st2ps,
            tc.tile_pool(name="f1ps", bufs=3, space="PSUM") as f1ps,
            tc.tile_pool(name="f2ps", bufs=2, space="PSUM") as f2ps,
        ):
            w2_sb = pb.tile([128, MT, FT, 128], BF, tag="w2")
            xres_sb = pb.tile([128, FT, RC], BF, tag="xres")
            nc.sync.dma_start(out=w2_sb[:], in_=d["w2"].ap())
            nc.sync.dma_start(out=xres_sb[:], in_=d["xres"].ap())
            h_sb = pb.tile([128, FT, RC], BF, tag="h")
            hn_sb = pb.tile([128, FT, RC], BF, tag="hn")
            a_sb = pb.tile([128, MT, RC], BF, tag="a")
            rs2_row = pb.tile([1, RC], BF, tag="rs2_row")
            nmu2_row = pb.tile([1, RC], BF, tag="nmu2_row")
            l2b = pb.tile([1, RC], BF, tag="l2b")
            l2a_b = pb.tile([128, RC], BF, tag="l2a_b")
            l2b_b = pb.tile([128, RC], BF, tag="l2b_b")

            nc.gpsimd.dma_start(out=h_sb[:], in_=hb[:])
            nc.vector.tensor_tensor(h_sb[:], h_sb[:], xres_sb[:], ALU.add)

            # LN2 stats (same tree + ones-matmul + row chain as LN1);
            # hn_sb doubles as the h^2 scratch until hn itself is written
            h2 = hn_sb
            t4b = sq2p.tile([128, 4, RC], BF, tag="t4b")
            sums2 = sq2p.tile([128, 2, RC], BF, tag="sums2")
            nc.vector.tensor_tensor(h2[:], h_sb[:], h_sb[:], ALU.mult)
            nc.vector.tensor_tensor(t4b[:], h_sb[:, 0:4], h_sb[:, 4:8], ALU.add)
            nc.vector.tensor_tensor(t4b[:, 0:2], t4b[:, 0:2], t4b[:, 2:4],
                                    ALU.add)
            nc.vector.tensor_tensor(sums2[:, 0], t4b[:, 0], t4b[:, 1], ALU.add)
            nc.vector.tensor_tensor(t4b[:], h2[:, 0:4], h2[:, 4:8], ALU.add)
            nc.vector.tensor_tensor(t4b[:, 0:2], t4b[:, 0:2], t4b[:, 2:4],
                                    ALU.add)
            nc.vector.tensor_tensor(sums2[:, 1], t4b[:, 0], t4b[:, 1], ALU.add)
            stp2 = st2ps.tile([1, 2, RC], F32, tag="stp2")
            nc.tensor.matmul(stp2[:, 0], onesp[:], sums2[:, 0],
                             start=True, stop=True)
            nc.tensor.matmul(stp2[:, 1], onesp[:], sums2[:, 1],
                             start=True, stop=True)
            vt2 = sq2p.tile([1, 2, RC], F32, tag="vt2")
            nc.vector.tensor_scalar(out=nmu2_row[:], in0=stp2[0:1, 0],
                                    scalar1=-1.0 / D, scalar2=None, op0=ALU.mult)
            nc.vector.tensor_tensor(vt2[:, 0], nmu2_row[:], nmu2_row[:],
                                    ALU.mult)
            nc.vector.tensor_scalar(out=vt2[:, 0], in0=vt2[:, 0], scalar1=EPS,
                                    scalar2=None, op0=ALU.subtract)
            nc.vector.scalar_tensor_tensor(vt2[:, 1], stp2[0:1, 1], 1.0 / D,
                                           vt2[:, 0], op0=ALU.mult,
                                           op1=ALU.subtract)
            nc.scalar.activation(vt2[:, 0], vt2[:, 1], AF.Ln)
            nc.scalar.activation(rs2_row[:], vt2[:, 0], AF.Exp, scale=-0.5)
            nc.vector.tensor_tensor(l2b[:], nmu2_row[:], rs2_row[:], ALU.mult)
            nc.gpsimd.partition_broadcast(l2a_b[:], rs2_row[:])
            nc.gpsimd.partition_broadcast(l2b_b[:], l2b[:])

            for f in range(FT):
                nc.vector.tensor_tensor(hn_sb[:, f], h_sb[:, f], l2a_b[:],
                                        ALU.mult)
                nc.vector.tensor_add(hn_sb[:, f], hn_sb[:, f], l2b_b[:])

            for m in range(MT):
                ps = f1ps.tile([128, RC], F32, tag="f1")
                for f in range(FT):
                    nc.tensor.matmul(ps[:], w1_sb[:, m, f], hn_sb[:, f],
                                     start=(f == 0), stop=(f == FT - 1))
                nc.scalar.activation(a_sb[:, m], ps[:], AF.Relu,
                                     bias=b1t[:, m:m + 1])

            for dd in range(FT):
                ps = f2ps.tile([128, RC], F32, tag="f2")
                for t in range(MT):
                    nc.tensor.matmul(ps[:], w2_sb[:, t, dd], a_sb[:, t],
                                     start=(t == 0), stop=(t == MT - 1))
                ot = outp.tile([128, RC], F32, tag="ot")
                nc.vector.scalar_tensor_tensor(
                    ot[:], ps[:], b2t[:, dd:dd + 1],
                    h_sb[:, dd], op0=ALU.add, op1=ALU.add)
                nc.scalar.dma_start(out=d["out"].ap()[:, dd], in_=ot[:])


# ----------------------------------------------------------------------
# host side
# ----------------------------------------------------------------------

BF_NP = ml_dtypes.bfloat16


def make_in_maps(x, mask, Wq, Wk, Wv, Wo, w1, b1, w2, b2, g1, be1, g2, be2):
    """Build the 8 per-core input maps from the full inputs."""
    f32 = np.float32
    x = np.asarray(x, f32)
    mask = np.asarray(mask)
    Wq, Wk, Wv, Wo = (np.asarray(t, f32) for t in (Wq, Wk, Wv, Wo))
    w1, b1, w2, b2 = (np.asarray(t, f32) for t in (w1, b1, w2, b2))
    g1, be1, g2, be2 = (np.asarray(t, f32) for t in (g1, be1, g2, be2))

    Wq_s = g1[:, None] * Wq / np.sqrt(np.float32(DH))
    Wk_s = g1[:, None] * Wk
    Wv_s = g1[:, None] * Wv
    bq_full = (be1 @ Wq) / np.sqrt(np.float32(DH))
    bk_full = be1 @ Wk
    bv_full = be1 @ Wv
    w1_s = g2[:, None] * w1
    b1_s = b1 + be2 @ w1
    m2d = np.asarray(mask[0, 0], bool)
    mask4 = np.stack([m2d[0:512, 128 * j:128 * j + 128].T.astype(f32)
                      for j in range(4)]).astype(BF_NP)
    onesp = np.ones((128, 1), BF_NP)
    ones64 = np.ones((128, 16, HC, 1), BF_NP)
    b1t = np.ascontiguousarray(b1_s.reshape(MT, 128).T).astype(f32)
    b2t = np.ascontiguousarray(b2.reshape(FT, 128).T).astype(f32)
    w1_p = np.ascontiguousarray(
        w1_s.reshape(FT, 128, MT, 128).transpose(1, 2, 0, 3)).astype(BF_NP)
    w2_p = np.ascontiguousarray(
        w2.reshape(MT, 128, FT, 128).transpose(1, 0, 2, 3)).astype(BF_NP)

    in_maps = []
    for c in range(N_CORES):
        g, r = divmod(c, TP)
        xT = np.ascontiguousarray(x[g].T)                       # [D, S]
        xt = np.ascontiguousarray(
            xT.reshape(FT, 128, S).transpose(1, 0, 2)).astype(BF_NP)
        xres = np.ascontiguousarray(
            xT[:, RC * r:RC * (r + 1)].reshape(FT, 128, RC)
            .transpose(1, 0, 2)).astype(BF_NP)
        sh = slice(DC * r, DC * (r + 1))
        wq_c = np.ascontiguousarray(
            Wq_s[:, sh].reshape(FT, 128, 2, 128).transpose(2, 1, 0, 3)
        ).astype(BF_NP)
        wk_c = np.ascontiguousarray(
            Wk_s[:, sh].reshape(FT, 128, 2, 128).transpose(2, 1, 0, 3)
        ).astype(BF_NP)
        wv_c = np.ascontiguousarray(
            Wv_s[:, sh].reshape(FT, 128, DC).transpose(1, 0, 2)).astype(BF_NP)
        wo_c = np.ascontiguousarray(
            Wo[sh, :].reshape(2, 128, FT, 128).transpose(1, 0, 2, 3)
        ).astype(BF_NP)
        qkvc = np.stack([Wq_s[:, sh].sum(0), Wk_s[:, sh].sum(0),
                         Wv_s[:, sh].sum(0), bq_full[sh], bk_full[sh],
                         bv_full[sh]]).astype(BF_NP)
        in_maps.append({
            "xt": xt, "xres": xres, "wq": wq_c, "wk": wk_c, "wv": wv_c,
            "qkvc": qkvc, "wo": wo_c, "w1": w1_p, "b1t": b1t, "w2": w2_p,
            "b2t": b2t, "mask4": mask4, "onesp": onesp, "ones64": ones64,
        })
    return in_maps


def assemble_output(results):
    """[8 x {out: [128, FT, RC]}] -> [B, S, D] float32."""
    out = np.empty((B, S, D), np.float32)
    for c in range(N_CORES):
        g, r = divmod(c, TP)
        ot = results[c]["out"].transpose(1, 0, 2).reshape(D, RC)  # [D, RC]
        out[g, RC * r:RC * (r + 1), :] = ot.T
    return out


_nc_cache = {}


def get_nc(repeat=1, qkv_bias=False):
    key = (repeat, qkv_bias)
    if key not in _nc_cache:
        _nc_cache[key] = build(repeat=repeat, qkv_bias=qkv_bias)
    return _nc_cache[key]


def kernel(**inputs):
    qkv_bias = bool(np.any(np.asarray(inputs["be1"], np.float32)))
    nc = get_nc(qkv_bias=qkv_bias)
    in_maps = make_in_maps(**inputs)
    res = run_bass_kernel_spmd(nc, in_maps, core_ids=list(range(N_CORES)))
    return assemble_output(res.results)


# revision 12
# speedup vs baseline: 1.4064x; 1.2884x over previous
"""Trainium2 Bass kernel: pre-norm decoder block (B=2, S=2048, D=1024, H=16, DFF=4096).

Sharding: 8 cores = 2 data-parallel groups (one per batch) x 4 tensor-parallel
ranks. Attention is head-sharded (4 heads/core, Megatron column-parallel QKV).
Each rank computes its partial Wo contribution (row-parallel Wo) per 512-wide
sequence block as attention for that block completes; a single bf16
ReduceScatter over the sequence axis then hands every rank the fully-reduced
pre-residual h for its own 512 rows. The rest (residual + LN2 + FFN +
residual) runs sequence-sharded with full w1/w2 (no further collectives).

Everything on-chip is bf16 (psum accumulation fp32): same tensor-engine speed
as fp32r at these tile sizes, but half the DMA/SBUF footprint and 2-4x DVE
throughput. LayerNorm statistics are computed by DVE pairwise-reduction trees
plus a single ones-vector matmul per 512-column block; the per-position LN
affine is folded into the projections via extra contraction rows (LN1) or a
broadcasted scale/shift (LN2).
"""

import numpy as np
import ml_dtypes

import concourse.bass as bass
import concourse.mybir as mybir
import concourse.tile as tile
from concourse import bacc
from concourse.bass_utils import run_bass_kernel_spmd

BF = mybir.dt.bfloat16
F32 = mybir.dt.float32
AF = mybir.ActivationFunctionType
ALU = mybir.AluOpType

B, S, D, H, DFF = 2, 2048, 1024, 16, 4096
DH = D // H
EPS = 1e-5

N_CORES = 8
TP = 4                    # tensor-parallel ranks per group
HC = H // TP              # heads per core
DC = HC * DH              # head features per core
RC = S // TP              # seq rows per core in stage B
FT = D // 128             # feature tiles
SB = S // 512             # 512-wide seq blocks
MT = DFF // 128           # dff tiles
REPLICA_GROUPS = [[0, 1, 2, 3], [4, 5, 6, 7]]


def build(repeat=1, qkv_bias=False, no_coll=False):
    nc = bacc.Bacc("TRN2", target_bir_lowering=False, debug=False,
                   num_devices=N_CORES)

    d = {"qkv_bias": qkv_bias, "no_coll": no_coll}
    d["xt"] = nc.dram_tensor("xt", [128, FT, S], BF, kind="ExternalInput")
    d["xres"] = nc.dram_tensor("xres", [128, FT, RC], BF, kind="ExternalInput")
    d["wq"] = nc.dram_tensor("wq", [2, 128, FT, 128], BF, kind="ExternalInput")
    d["wk"] = nc.dram_tensor("wk", [2, 128, FT, 128], BF, kind="ExternalInput")
    d["wv"] = nc.dram_tensor("wv", [128, FT, DC], BF, kind="ExternalInput")
    d["qkvc"] = nc.dram_tensor("qkvc", [6, DC], BF, kind="ExternalInput")
    d["wo"] = nc.dram_tensor("wo", [128, 2, FT, 128], BF, kind="ExternalInput")
    d["w1"] = nc.dram_tensor("w1", [128, MT, FT, 128], BF, kind="ExternalInput")
    d["b1t"] = nc.dram_tensor("b1t", [128, MT], F32, kind="ExternalInput")
    d["w2"] = nc.dram_tensor("w2", [128, MT, FT, 128], BF, kind="ExternalInput")
    d["b2t"] = nc.dram_tensor("b2t", [128, FT], F32, kind="ExternalInput")
    d["mask4"] = nc.dram_tensor("mask4", [4, 128, 512], BF, kind="ExternalInput")
    d["onesp"] = nc.dram_tensor("onesp", [128, 1], BF, kind="ExternalInput")
    d["ones64"] = nc.dram_tensor("ones64", [128, 16, HC, 1], BF, kind="ExternalInput")
    d["out"] = nc.dram_tensor("out", [128, FT, RC], F32, kind="ExternalOutput")

    with tile.TileContext(nc) as tc:
        for _ in range(repeat):
            _emit(nc, tc, d)

    nc.compile()
    return nc


def _emit(nc, tc, d):
    qkv_bias = d["qkv_bias"]
    with (
        tc.tile_pool(name="dram", bufs=1, space="DRAM") as dramp,
        tc.tile_pool(name="const", bufs=1) as cp,
        tc.tile_pool(name="w1p", bufs=1) as w1p,
    ):
        # DRAM bounce buffers for the collective
        wob = dramp.tile([SB, 128, FT, RC], BF, tag="wob")
        hb = dramp.tile([128, FT, RC], BF, tag="hb")
        drs = dramp.tile([1, S], F32, tag="drs")

        # persistent small tensors + weight prefetch (no dependencies)
        onesp = cp.tile([128, 1], BF, tag="onesp")
        b1t = cp.tile([128, MT], F32, tag="b1t")
        b2t = cp.tile([128, FT], F32, tag="b2t")
        rst = cp.tile([128, 16], F32, tag="rst")
        ncst = 6 if qkv_bias else 3
        cst = [cp.tile([1, DC], BF, tag=f"qkvc{i}", name=f"qkvc{i}")
               for i in range(ncst)]
        w1_sb = w1p.tile([128, MT, FT, 128], BF, tag="w1")

        nc.sync.dma_start(out=onesp[:], in_=d["onesp"].ap())
        for i in range(ncst):
            nc.sync.dma_start(out=cst[i][:], in_=d["qkvc"].ap()[i:i + 1, :])
        nc.sync.dma_start(out=b1t[:], in_=d["b1t"].ap())
        nc.sync.dma_start(out=b2t[:], in_=d["b2t"].ap())
        wqs, wks, wvs = cst[0][:], cst[1][:], cst[2][:]
        if qkv_bias:
            bqc, bkc, bvc = cst[3][:], cst[4][:], cst[5][:]

        with tc.tile_pool(name="persa", bufs=1) as pa:
            qt_sb = pa.tile([128, 2, S], BF, tag="qt")
            kt_sb = pa.tile([128, 2, S], BF, tag="kt")
            v_sb = pa.tile([128, 16, HC, 65], BF, tag="v")
            mask4 = pa.tile([128, 4, 512], BF, tag="mask4")
            wo_sb = pa.tile([128, 2, FT, 128], BF, tag="wo")
            nc.sync.dma_start(out=mask4[:],
                              in_=d["mask4"].ap().rearrange("j p c -> p j c"))

            # ================= phase 1: LN1 stats + projections ==========
            with (
                tc.tile_pool(name="xpool", bufs=1) as xp,
                tc.tile_pool(name="sq", bufs=1) as sqp,
                tc.tile_pool(name="vtp", bufs=2) as vtp,
                tc.tile_pool(name="stps", bufs=2, space="PSUM") as stps,
                tc.tile_pool(name="prps", bufs=3, space="PSUM") as prps,
            ):
                x_sb = xp.tile([128, FT, S], BF, tag="x")
                wq_sb = xp.tile([128, 2, FT, 128], BF, tag="wq")
                wk_sb = xp.tile([128, 2, FT, 128], BF, tag="wk")
                wv_sb = xp.tile([128, FT, DC], BF, tag="wv")
                nmu_row = xp.tile([1, S], BF, tag="nmu_row")
                std_row = xp.tile([1, S], BF, tag="std_row") if qkv_bias else None
                rs_row = xp.tile([1, S], F32, tag="rs_row")
                a_b = xp.tile([128, S], F32, tag="a_b")

                # x first (critical path), then stage-A weights, then
                # stage-B weights (wo tiny, w1 big) — all on the sync queue.
                for b in range(SB):
                    sl = bass.ts(b, 512)
                    nc.sync.dma_start(out=x_sb[:, :, sl],
                                      in_=d["xt"].ap()[:, :, sl])
                for dd in range(2):
                    nc.sync.dma_start(out=wq_sb[:, dd], in_=d["wq"].ap()[dd])
                    nc.sync.dma_start(out=wk_sb[:, dd], in_=d["wk"].ap()[dd])
                nc.sync.dma_start(out=wv_sb[:], in_=d["wv"].ap())
                nc.sync.dma_start(out=v_sb[:, :, :, 64:65], in_=d["ones64"].ap())
                nc.sync.dma_start(out=wo_sb[:], in_=d["wo"].ap())
                nc.sync.dma_start(out=w1_sb[:], in_=d["w1"].ap())

                def emit_stats(b):
                    sl = bass.ts(b, 512)
                    # DVE pairwise trees for sum(x) and sum(x^2) over the
                    # 8 feature tiles, then one ones-vector matmul for the
                    # partition reduction.
                    x2 = sqp.tile([128, FT, 512], BF, tag="x2")
                    t4 = sqp.tile([128, 4, 512], BF, tag="t4")
                    sums = sqp.tile([128, 2, 512], BF, tag="sums")
                    nc.vector.tensor_tensor(x2[:], x_sb[:, :, sl], x_sb[:, :, sl],
                                            ALU.mult)
                    nc.vector.tensor_tensor(t4[:], x_sb[:, 0:4, sl],
                                            x_sb[:, 4:8, sl], ALU.add)
                    nc.vector.tensor_tensor(t4[:, 0:2], t4[:, 0:2], t4[:, 2:4],
                                            ALU.add)
                    nc.vector.tensor_tensor(sums[:, 0], t4[:, 0], t4[:, 1],
                                            ALU.add)
                    nc.vector.tensor_tensor(t4[:], x2[:, 0:4], x2[:, 4:8],
                                            ALU.add)
                    nc.vector.tensor_tensor(t4[:, 0:2], t4[:, 0:2], t4[:, 2:4],
                                            ALU.add)
                    nc.vector.tensor_tensor(sums[:, 1], t4[:, 0], t4[:, 1],
                                            ALU.add)
                    stp = stps.tile([1, 2, 512], F32, tag="stp")
                    nc.tensor.matmul(stp[:, 0], onesp[:], sums[:, 0],
                                     start=True, stop=True)
                    nc.tensor.matmul(stp[:, 1], onesp[:], sums[:, 1],
                                     start=True, stop=True)
                    # row chain: nmu = -sum(x)/D ; var = sum(x2)/D - mu^2 ;
                    # rs = exp(-0.5*ln(var+eps)) ; a_b = broadcast(rs)
                    vt = vtp.tile([1, 2, 512], F32, tag="vt")
                    nc.vector.tensor_scalar(out=nmu_row[:, sl], in0=stp[0:1, 0],
                                            scalar1=-1.0 / D, scalar2=None,
                                            op0=ALU.mult)
                    nc.vector.tensor_tensor(vt[:, 0], nmu_row[:, sl],
                                            nmu_row[:, sl], ALU.mult)
                    nc.vector.tensor_scalar(out=vt[:, 0], in0=vt[:, 0],
                                            scalar1=EPS, scalar2=None,
                                            op0=ALU.subtract)
                    nc.vector.scalar_tensor_tensor(vt[:, 1], stp[0:1, 1],
                                                   1.0 / D, vt[:, 0],
                                                   op0=ALU.mult,
                                                   op1=ALU.subtract)
                    nc.scalar.activation(vt[:, 0], vt[:, 1], AF.Ln)
                    nc.scalar.activation(rs_row[:, sl], vt[:, 0], AF.Exp,
                                         scale=-0.5)
                    if qkv_bias:
                        nc.scalar.activation(std_row[:, sl], vt[:, 0], AF.Exp,
                                             scale=0.5)
                    nc.gpsimd.partition_broadcast(a_b[:, sl], rs_row[:, sl])
                    # rs for v-scaling needs a [128, 4] column layout: bounce
                    # through DRAM with a rearranging read.
                    nc.scalar.dma_start(out=drs[:, sl], in_=rs_row[:, sl])
                    nc.gpsimd.dma_start(
                        out=rst[:, 4 * b:4 * b + 4],
                        in_=drs[:, sl].rearrange("o (t p) -> (o p) t", p=128))

                def emit_proj(b):
                    sl = bass.ts(b, 512)
                    for (w_sb, wsum, bc, o_sb) in (
                            (wq_sb, wqs, bqc if qkv_bias else None, qt_sb),
                            (wk_sb, wks, bkc if qkv_bias else None, kt_sb)):
                        for dd in range(2):
                            dsl = bass.ts(dd, 128)
                            ps = prps.tile([128, 512], F32, tag="pp")
                            for f in range(FT):
                                nc.tensor.matmul(ps[:], w_sb[:, dd, f],
                                                 x_sb[:, f, sl],
                                                 start=(f == 0), stop=False)
                            nc.tensor.matmul(ps[:], wsum[0:1, dsl],
                                             nmu_row[:, sl], start=False,
                                             stop=not qkv_bias)
                            if qkv_bias:
                                nc.tensor.matmul(ps[:], bc[0:1, dsl],
                                                 std_row[:, sl],
                                                 start=False, stop=True)
                            nc.vector.tensor_tensor(o_sb[:, dd, sl], ps[:],
                                                    a_b[:, sl], ALU.mult)
                    for i in range(4 * b, 4 * b + 4):
                        rl = bass.ts(i, 128)
                        ps = prps.tile([128, 512], F32, tag="pp")
                        for f in range(FT):
                            nc.tensor.matmul(ps[:, 0:DC], x_sb[:, f, rl],
                                             wv_sb[:, f],
                                             start=(f == 0), stop=False)
                        nc.tensor.matmul(ps[:, 0:DC], nmu_row[:, rl], wvs,
                                         start=False, stop=not qkv_bias)
                        if qkv_bias:
                            nc.tensor.matmul(ps[:, 0:DC], std_row[:, rl], bvc,
                                             start=False, stop=True)
                        nc.vector.tensor_scalar(
                            out=v_sb[:, i, :, 0:64],
                            in0=ps[:, 0:DC].rearrange("p (h e) -> p h e", h=HC),
                            scalar1=rst[:, i:i + 1], scalar2=None,
                            op0=ALU.mult)

                # software-pipelined by one block so the DVE never
                # head-of-line blocks the next block's stats tree
                emit_stats(0)
                for b in range(1, SB):
                    emit_stats(b)
                    emit_proj(b - 1)
                emit_proj(SB - 1)
            # xpool closed

            # ================= phase 2: attention + Wo partials ==========
            with (
                tc.tile_pool(name="atq", bufs=2) as atqp,
                tc.tile_pool(name="wos", bufs=2) as wosp,
                tc.tile_pool(name="exps", bufs=8) as expp,
                tc.tile_pool(name="rcps", bufs=4) as rcpp,
                tc.tile_pool(name="scps", bufs=4, space="PSUM") as scps,
                tc.tile_pool(name="pvps", bufs=4, space="PSUM") as pvps,
            ):
                for qi in range(SB):
                    qsl = bass.ts(qi, 512)
                    attn_qi = atqp.tile([128, 2, 512], BF, tag="attq")
                    for hp in range(2):
                        heads = (2 * hp, 2 * hp + 1)
                        nki = 4 * qi + 4
                        pv = {h: pvps.tile([65, 512], F32, tag="pv",
                                           name=f"pv{h}_{qi}") for h in heads}
                        for ki in range(nki):
                            for h in heads:
                                hb_ = 64 * (h % 2)
                                hs = slice(hb_, hb_ + 64)
                                sc = scps.tile([128, 512], F32, tag="sc")
                                nc.tensor.matmul(
                                    sc[:], kt_sb[hs, hp, bass.ts(ki, 128)],
                                    qt_sb[hs, hp, qsl], start=True, stop=True)
                                ex = expp.tile([128, 512], BF, tag="ex")
                                nc.scalar.activation(ex[:], sc[:], AF.Exp)
                                rel = 128 * ki - 512 * qi
                                if rel >= 0:
                                    mw = rel + 128
                                    nc.vector.tensor_tensor(
                                        ex[:, 0:mw], ex[:, 0:mw],
                                        mask4[:, rel // 128, 0:mw], ALU.mult)
                                nc.tensor.matmul(pv[h][:], v_sb[:, ki, h, :],
                                                 ex[:], start=(ki == 0),
                                                 stop=(ki == nki - 1))
                        for h in heads:
                            hb_ = 64 * (h % 2)
                            rcp = rcpp.tile([1, 512], BF, tag="rcp")
                            rcpb = rcpp.tile([64, 512], BF, tag="rcpb")
                            with nc.allow_low_precision(reason="bf16 softmax"):
                                nc.vector.reciprocal(rcp[:], pv[h][64:65, :])
                            nc.gpsimd.partition_broadcast(rcpb[:], rcp[:])
                            nc.vector.tensor_tensor(
                                attn_qi[hb_:hb_ + 64, hp, :],
                                pv[h][0:64, :], rcpb[:], ALU.mult)
                    # row-parallel Wo partial for this seq block -> bounce
                    wops = wosp.tile([128, FT, 512], BF, tag="wops")
                    for dd in range(FT):
                        ps = scps.tile([128, 512], F32, tag="sc")
                        nc.tensor.matmul(ps[:], wo_sb[:, 0, dd],
                                         attn_qi[:, 0, :],
                                         start=True, stop=False)
                        nc.tensor.matmul(ps[:], wo_sb[:, 1, dd],
                                         attn_qi[:, 1, :],
                                         start=False, stop=True)
                        nc.vector.tensor_copy(out=wops[:, dd, :], in_=ps[:])
                    nc.gpsimd.dma_start(out=wob[qi], in_=wops[:])

                # one bf16 ReduceScatter over the seq axis: rank r receives
                # sum_ranks(partial h) for its own 512 rows
                if d.get("no_coll"):
                    nc.gpsimd.dma_start(out=hb[:], in_=wob[0])
                else:
                    nc.gpsimd.collective_compute(
                        "ReduceScatter", ALU.add, replica_groups=REPLICA_GROUPS,
                        ins=[wob[:].opt()], outs=[hb[:].opt()])
        # persa closed

        # ================= phase 3: residual + LN2 + FFN =================
        with (
            tc.tile_pool(name="pb", bufs=1) as pb,
            tc.tile_pool(name="sq2", bufs=1) as sq2p,
            tc.tile_pool(name="outp", bufs=2) as outp,
            tc.tile_pool(name="st2ps", bufs=1, space="PSUM") as st2ps,
            tc.tile_pool(name="f1ps", bufs=3, space="PSUM") as f1ps,
            tc.tile_pool(name="f2ps", bufs=2, space="PSUM") as f2ps,
        ):
            w2_sb = pb.tile([128, MT, FT, 128], BF, tag="w2")
            xres_sb = pb.tile([128, FT, RC], BF, tag="xres")
            nc.sync.dma_start(out=w2_sb[:], in_=d["w2"].ap())
            nc.sync.dma_start(out=xres_sb[:], in_=d["xres"].ap())
            h_sb = pb.tile([128, FT, RC], BF, tag="h")
            hn_sb = pb.tile([128, FT, RC], BF, tag="hn")
            a_sb = pb.tile([128, MT, RC], BF, tag="a")
            rs2_row = pb.tile([1, RC], BF, tag="rs2_row")
            nmu2_row = pb.tile([1, RC], BF, tag="nmu2_row")
            l2b = pb.tile([1, RC], BF, tag="l2b")
            l2a_b = pb.tile([128, RC], BF, tag="l2a_b")
            l2b_b = pb.tile([128, RC], BF, tag="l2b_b")

            nc.gpsimd.dma_start(out=h_sb[:], in_=hb[:])
            nc.vector.tensor_tensor(h_sb[:], h_sb[:], xres_sb[:], ALU.add)

            # LN2 stats (same tree + ones-matmul + row chain as LN1);
            # hn_sb doubles as the h^2 scratch until hn itself is written
            h2 = hn_sb
            t4b = sq2p.tile([128, 4, RC], BF, tag="t4b")
            sums2 = sq2p.tile([128, 2, RC], BF, tag="sums2")
            nc.vector.tensor_tensor(h2[:], h_sb[:], h_sb[:], ALU.mult)
            nc.vector.tensor_tensor(t4b[:], h_sb[:, 0:4], h_sb[:, 4:8], ALU.add)
            nc.vector.tensor_tensor(t4b[:, 0:2], t4b[:, 0:2], t4b[:, 2:4],
                                    ALU.add)
            nc.vector.tensor_tensor(sums2[:, 0], t4b[:, 0], t4b[:, 1], ALU.add)
            nc.vector.tensor_tensor(t4b[:], h2[:, 0:4], h2[:, 4:8], ALU.add)
            nc.vector.tensor_tensor(t4b[:, 0:2], t4b[:, 0:2], t4b[:, 2:4],
                                    ALU.add)
            nc.vector.tensor_tensor(sums2[:, 1], t4b[:, 0], t4b[:, 1], ALU.add)
            stp2 = st2ps.tile([1, 2, RC], F32, tag="stp2")
            nc.tensor.matmul(stp2[:, 0], onesp[:], sums2[:, 0],
                             start=True, stop=True)
            nc.tensor.matmul(stp2[:, 1], onesp[:], sums2[:, 1],
                             start=True, stop=True)
            vt2 = sq2p.tile([1, 2, RC], F32, tag="vt2")
            nc.vector.tensor_scalar(out=nmu2_row[:], in0=stp2[0:1, 0],
                                    scalar1=-1.0 / D, scalar2=None, op0=ALU.mult)
            nc.vector.tensor_tensor(vt2[:, 0], nmu2_row[:], nmu2_row[:],
                                    ALU.mult)
            nc.vector.tensor_scalar(out=vt2[:, 0], in0=vt2[:, 0], scalar1=EPS,
                                    scalar2=None, op0=ALU.subtract)
            nc.vector.scalar_tensor_tensor(vt2[:, 1], stp2[0:1, 1], 1.0 / D,
                                           vt2[:, 0], op0=ALU.mult,
                                           op1=ALU.subtract)
            nc.scalar.activation(vt2[:, 0], vt2[:, 1], AF.Ln)
            nc.scalar.activation(rs2_row[:], vt2[:, 0], AF.Exp, scale=-0.5)
            nc.vector.tensor_tensor(l2b[:], nmu2_row[:], rs2_row[:], ALU.mult)
            nc.gpsimd.partition_broadcast(l2a_b[:], rs2_row[:])
            nc.gpsimd.partition_broadcast(l2b_b[:], l2b[:])

            for f in range(FT):
                nc.vector.tensor_tensor(hn_sb[:, f], h_sb[:, f], l2a_b[:],
                                        ALU.mult)
                nc.vector.tensor_add(hn_sb[:, f], hn_sb[:, f], l2b_b[:])

            for m in range(MT):
                ps = f1ps.tile([128, RC], F32, tag="f1")
                for f in range(FT):
                    nc.tensor.matmul(ps[:], w1_sb[:, m, f], hn_sb[:, f],
                                     start=(f == 0), stop=(f == FT - 1))
                nc.scalar.activation(a_sb[:, m], ps[:], AF.Relu,
                                     bias=b1t[:, m:m + 1])

            for dd in range(FT):
                ps = f2ps.tile([128, RC], F32, tag="f2")
                for t in range(MT):
                    nc.tensor.matmul(ps[:], w2_sb[:, t, dd], a_sb[:, t],
                                     start=(t == 0), stop=(t == MT - 1))
                ot = outp.tile([128, RC], F32, tag="ot")
                nc.vector.scalar_tensor_tensor(
                    ot[:], ps[:], b2t[:, dd:dd + 1],
                    h_sb[:, dd], op0=ALU.add, op1=ALU.add)
                nc.scalar.dma_start(out=d["out"].ap()[:, dd], in_=ot[:])


# ----------------------------------------------------------------------
# host side
# ----------------------------------------------------------------------

BF_NP = ml_dtypes.bfloat16


def make_in_maps(x, mask, Wq, Wk, Wv, Wo, w1, b1, w2, b2, g1, be1, g2, be2):
    """Build the 8 per-core input maps from the full inputs."""
    f32 = np.float32
    x = np.asarray(x, f32)
    mask = np.asarray(mask)
    Wq, Wk, Wv, Wo = (np.asarray(t, f32) for t in (Wq, Wk, Wv, Wo))
    w1, b1, w2, b2 = (np.asarray(t, f32) for t in (w1, b1, w2, b2))
    g1, be1, g2, be2 = (np.asarray(t, f32) for t in (g1, be1, g2, be2))

    Wq_s = g1[:, None] * Wq / np.sqrt(np.float32(DH))
    Wk_s = g1[:, None] * Wk
    Wv_s = g1[:, None] * Wv
    bq_full = (be1 @ Wq) / np.sqrt(np.float32(DH))
    bk_full = be1 @ Wk
    bv_full = be1 @ Wv
    w1_s = g2[:, None] * w1
    b1_s = b1 + be2 @ w1
    m2d = np.asarray(mask[0, 0], bool)
    mask4 = np.stack([m2d[0:512, 128 * j:128 * j + 128].T.astype(f32)
                      for j in range(4)]).astype(BF_NP)
    onesp = np.ones((128, 1), BF_NP)
    ones64 = np.ones((128, 16, HC, 1), BF_NP)
    b1t = np.ascontiguousarray(b1_s.reshape(MT, 128).T).astype(f32)
    b2t = np.ascontiguousarray(b2.reshape(FT, 128).T).astype(f32)
    w1_p = np.ascontiguousarray(
        w1_s.reshape(FT, 128, MT, 128).transpose(1, 2, 0, 3)).astype(BF_NP)
    w2_p = np.ascontiguousarray(
        w2.reshape(MT, 128, FT, 128).transpose(1, 0, 2, 3)).astype(BF_NP)

    in_maps = []
    for c in range(N_CORES):
        g, r = divmod(c, TP)
        xT = np.ascontiguousarray(x[g].T)                       # [D, S]
        xt = np.ascontiguousarray(
            xT.reshape(FT, 128, S).transpose(1, 0, 2)).astype(BF_NP)
        xres = np.ascontiguousarray(
            xT[:, RC * r:RC * (r + 1)].reshape(FT, 128, RC)
            .transpose(1, 0, 2)).astype(BF_NP)
        sh = slice(DC * r, DC * (r + 1))
        wq_c = np.ascontiguousarray(
            Wq_s[:, sh].reshape(FT, 128, 2, 128).transpose(2, 1, 0, 3)
        ).astype(BF_NP)
        wk_c = np.ascontiguousarray(
            Wk_s[:, sh].reshape(FT, 128, 2, 128).transpose(2, 1, 0, 3)
        ).astype(BF_NP)
        wv_c = np.ascontiguousarray(
            Wv_s[:, sh].reshape(FT, 128, DC).transpose(1, 0, 2)).astype(BF_NP)
        wo_c = np.ascontiguousarray(
            Wo[sh, :].reshape(2, 128, FT, 128).transpose(1, 0, 2, 3)
        ).astype(BF_NP)
        qkvc = np.stack([Wq_s[:, sh].sum(0), Wk_s[:, sh].sum(0),
                         Wv_s[:, sh].sum(0), bq_full[sh], bk_full[sh],
                         bv_full[sh]]).astype(BF_NP)
        in_maps.append({
            "xt": xt, "xres": xres, "wq": wq_c, "wk": wk_c, "wv": wv_c,
            "qkvc": qkvc, "wo": wo_c, "w1": w1_p, "b1t": b1t, "w2": w2_p,
            "b2t": b2t, "mask4": mask4, "onesp": onesp, "ones64": ones64,
        })
    return in_maps


def assemble_output(results):
    """[8 x {out: [128, FT, RC]}] -> [B, S, D] float32."""
    out = np.empty((B, S, D), np.float32)
    for c in range(N_CORES):
        g, r = divmod(c, TP)
        ot = results[c]["out"].transpose(1, 0, 2).reshape(D, RC)  # [D, RC]
        out[g, RC * r:RC * (r + 1), :] = ot.T
    return out


_nc_cache = {}


def get_nc(repeat=1, qkv_bias=False, no_coll=False):
    key = (repeat, qkv_bias, no_coll)
    if key not in _nc_cache:
        _nc_cache[key] = build(repeat=repeat, qkv_bias=qkv_bias,
                               no_coll=no_coll)
    return _nc_cache[key]


def kernel(**inputs):
    qkv_bias = bool(np.any(np.asarray(inputs["be1"], np.float32)))
    nc = get_nc(qkv_bias=qkv_bias)
    in_maps = make_in_maps(**inputs)
    res = run_bass_kernel_spmd(nc, in_maps, core_ids=list(range(N_CORES)))
    return assemble_output(res.results)
